# revision 2
# baseline (speedup 1.0000x reference)
"""AttentionCrop Trainium2 kernel v3 (8 NeuronCores, data-parallel over batch).

Math (verified 0-wrong):  s = row-sum(mask), l_eff = max(l, s/2),
  a = t - l_eff, e = min(t + l_eff, s - 0.5); out[j] = (a <= j < e).

Measured-rate design (per [128, 4096] tile):
  T1 (tiles 0-5, u8 out):  ACT Square halves (2x 1.89us, fp16 idx input,
      per-partition bias 2048k - c) -> sq f32; DVE tensor_scalar
      is_le(sq, h|h|*1.0000003 + 0.2) -> u8 at 2x mode (2.26us).
  T3 (tiles 6-7, fp16 out): DVE per 2048-block: two fp16 4x compares
      (0.66us each) + one full-tile fp16 tensor_tensor mult at 2x (2.26us).
  Empty rows (a >= e): h < 0 so h|h| < 0 >= nothing passes; fp16 path
      compares fail likewise.
  s recovery: strided probes mask[:, 512k] k=2..7 -> c = ceil(s/512);
  512-wide SWDGE window at chunk c-1 -> exact remainder.  4 groups of
  2 tiles pipeline probes -> gather -> window-sum -> chain -> binarize.
  Window sums ride ACT Square accum_out; chains are small [128,2] DVE ops.
  Stores issue from the DVE queue (sync queue holds only input DMAs).
"""

import sys

import numpy as np

if "/opt/trn_rl_repo" not in sys.path:
    sys.path.insert(0, "/opt/trn_rl_repo")

import concourse.bacc as bacc
import concourse.bass as bass
import concourse.mybir as mybir
import concourse.tile as tile
from concourse.bass_utils import run_bass_kernel_spmd

N_CORES = 8
B, L = 8192, 4096
ROWS = B // N_CORES
NT = ROWS // 128
PROBE = 512
NPROBE = L // PROBE
KMIN = 2
NPR = NPROBE - KMIN
HB = 2048

F32 = mybir.dt.float32
F16 = mybir.dt.float16
U8 = mybir.dt.uint8
I32 = mybir.dt.int32
A = mybir.AluOpType
AF = mybir.ActivationFunctionType

# pairs pipeline; T3 tiles first (pure-DVE, no ACT dependency), then T1
PAIRS = ((6, 7), (0, 1), (2, 3), (4, 5))
T1_TILES = (0, 1, 2, 3, 4, 5)      # u8 out
T3_TILES = (6, 7)                  # fp16 out


def build_bass() -> bass.Bass:
    nc = bacc.Bacc()
    m_in = nc.declare_dram_parameter("mask", [ROWS, L], F32, isOutput=False)
    aux_in = nc.declare_dram_parameter("aux", [128, 3 * NT], F32, isOutput=False)
    idxh_in = nc.declare_dram_parameter("idxh", [128, HB], F16, isOutput=False)
    outb_d = nc.declare_dram_parameter("outb", [len(T1_TILES) * 128, L], U8,
                                       isOutput=True)
    outh_d = nc.declare_dram_parameter("outh", [len(T3_TILES) * 128, L], F16,
                                       isOutput=True)

    m_chunks = m_in.rearrange("r (k s) -> (r k) s", s=PROBE)
    m_probes = m_in.rearrange("(q p) (k s) -> p q k s", p=128, s=PROBE)

    with tile.TileContext(nc) as tc:
        with (
            tc.tile_pool(name="const", bufs=1) as cpool,
            tc.tile_pool(name="big", bufs=1) as bpool,
            tc.tile_pool(name="sq", bufs=6) as qpool,
            tc.tile_pool(name="small", bufs=1) as spool,
        ):
            aux = cpool.tile([128, 3 * NT], F32, tag="aux")
            t8 = aux[:, 0:NT]
            l8 = aux[:, NT:2 * NT]
            cb8 = aux[:, 2 * NT:3 * NT]

            idx_h = cpool.tile([128, HB], F16, tag="idxh")
            pr = cpool.tile([128, NT * NPR], F32, tag="pr")
            win = cpool.tile([128, NT * PROBE], F32, tag="win")
            junk = cpool.tile([128, PROBE], F32, tag="junk")
            tmp1 = cpool.tile([128, L], F16, tag="t3tmp", name="t3tmp")

            def small(tag, w=2, dt=F32):
                return spool.tile([128, w], dt, tag=tag, name=tag)

            prm = {g: {} for g in range(4)}

            # ---------------- emission ----------------
            # input DMAs on sync: probes pair-by-pair, aux + idx early
            for g, (q0, q1) in enumerate(PAIRS):
                eng = nc.sync if g < 2 else nc.scalar
                for q in (q0, q1):
                    eng.dma_start(
                        pr[:, q * NPR:(q + 1) * NPR],
                        m_probes[:, q, KMIN:NPROBE, 0],
                    )
                if g == 0:
                    nc.sync.dma_start(aux[:], aux_in[:, :])
                    nc.sync.dma_start(idx_h[:], idxh_in[:, :])

            def front(g):
                q0, q1 = PAIRS[g]
                cnt = small(f"cnt{g}")
                nc.vector.tensor_reduce(
                    cnt[:],
                    pr[:, q0 * NPR:(q1 + 1) * NPR].rearrange(
                        "p (q k) -> p q k", k=NPR),
                    axis=mybir.AxisListType.X, op=A.add,
                )
                wi = small(f"wi{g}", dt=I32)
                nc.vector.scalar_tensor_tensor(
                    wi[:], cnt[:], 1.0, cb8[:, q0:q1 + 1], A.add, A.add)
                for j, q in enumerate((q0, q1)):
                    nc.gpsimd.indirect_dma_start(
                        out=win[:, q * PROBE:(q + 1) * PROBE],
                        out_offset=None,
                        in_=m_chunks,
                        in_offset=bass.IndirectOffsetOnAxis(
                            ap=wi[:, j:j + 1], axis=0),
                    )
                prm[g]["cnt"] = cnt

            def chain(g, rem_engine="act"):
                q0, q1 = PAIRS[g]
                v = nc.vector
                rem = small(f"rem{g}")
                if rem_engine == "act":
                    for j, q in enumerate((q0, q1)):
                        nc.scalar.activation(
                            junk[:], win[:, q * PROBE:(q + 1) * PROBE],
                            AF.Square, accum_out=rem[:, j:j + 1])
                else:
                    v.tensor_reduce(
                        rem[:],
                        win[:, q0 * PROBE:(q1 + 1) * PROBE].rearrange(
                            "p (w s) -> p w s", s=PROBE),
                        axis=mybir.AxisListType.X, op=A.add,
                    )
                cnt = prm[g]["cnt"]
                sr = small(f"sr{g}")     # s - 512
                v.scalar_tensor_tensor(sr[:], cnt[:], float(PROBE), rem[:],
                                       A.mult, A.add)
                sh = small(f"sh{g}")     # s/2
                v.tensor_scalar(sh[:], sr[:], 0.5, float(PROBE) * 0.5,
                                A.mult, A.add)
                le = small(f"le{g}")
                v.tensor_tensor(le[:], sh[:], l8[:, q0:q1 + 1], A.max)
                a = small(f"a{g}")
                v.tensor_tensor(a[:], t8[:, q0:q1 + 1], le[:], A.subtract)
                e0 = small(f"e0{g}")
                v.tensor_tensor(e0[:], t8[:, q0:q1 + 1], le[:], A.add)
                sm = small(f"sm{g}")     # s - 0.5
                v.tensor_scalar(sm[:], sr[:], float(PROBE) - 0.5, None, A.add)
                e = small(f"e{g}")
                v.tensor_tensor(e[:], e0[:], sm[:], A.min)
                prm[g].update(a=a, e=e)
                if PAIRS[g][0] in T1_TILES:
                    # center form: negc_k = 2048k - c, hsm = h|h|*(1+3e-7)+0.2
                    cs = small(f"cs{g}")
                    v.tensor_tensor(cs[:], a[:], e[:], A.add)
                    negc = small(f"negc{g}")
                    v.tensor_scalar(negc[:], cs[:], -0.5, None, A.mult)
                    negc2 = small(f"negc2{g}")
                    v.tensor_scalar(negc2[:], negc[:], float(HB), None, A.add)
                    dd = small(f"dd{g}")
                    v.tensor_tensor(dd[:], e[:], a[:], A.subtract)
                    h = small(f"h{g}")
                    v.tensor_scalar(h[:], dd[:], 0.5, None, A.mult)
                    hneg = small(f"hneg{g}")
                    v.tensor_scalar(hneg[:], h[:], -1.0, None, A.mult)
                    habs = small(f"habs{g}")
                    v.tensor_tensor(habs[:], h[:], hneg[:], A.max)
                    hh = small(f"hh{g}")
                    v.tensor_tensor(hh[:], h[:], habs[:], A.mult)
                    hsm = small(f"hsm{g}")
                    v.tensor_scalar(hsm[:], hh[:], 1.0000003, 0.2,
                                    A.mult, A.add)
                    prm[g].update(negc=negc, negc2=negc2, hsm=hsm)

            def t1_sq(g, j, q):
                p = prm[g]
                sq = qpool.tile([128, L], F32, tag="sq", name=f"sq_{q}")
                nc.scalar.activation(sq[:, 0:HB], idx_h[:], AF.Square,
                                     bias=p["negc"][:, j:j + 1])
                nc.scalar.activation(sq[:, HB:L], idx_h[:], AF.Square,
                                     bias=p["negc2"][:, j:j + 1])
                return sq

            def t1_isle(g, j, q, sq, split_store=False):
                p = prm[g]
                bu = bpool.tile([128, L], U8, tag=f"b{q}", name=f"b_{q}")
                base = T1_TILES.index(q) * 128
                for k in range(2):
                    nc.vector.tensor_scalar(
                        bu[:, k * HB:(k + 1) * HB], sq[:, k * HB:(k + 1) * HB],
                        p["hsm"][:, j:j + 1], None, A.is_le)
                    if split_store:
                        nc.sync.dma_start(
                            outb_d[base:base + 128, k * HB:(k + 1) * HB],
                            bu[:, k * HB:(k + 1) * HB])
                if not split_store:
                    nc.sync.dma_start(outb_d[base:base + 128, :], bu[:])

            def t3_tile(g, j, q):
                p = prm[g]
                a = p["a"][:, j:j + 1]
                e = p["e"][:, j:j + 1]
                bh = bpool.tile([128, L], F16, tag=f"b{q}", name=f"b_{q}")
                for k in range(2):
                    # tmp1 block: j >= a ; bh block: j < e (both fp16 4x)
                    nc.vector.tensor_scalar(
                        tmp1[:, k * HB:(k + 1) * HB], idx_h[:], a,
                        -float(HB) * k, A.subtract, A.is_ge)
                    nc.vector.tensor_scalar(
                        bh[:, k * HB:(k + 1) * HB], idx_h[:], e,
                        -float(HB) * k, A.subtract, A.is_lt)
                # combine full tile (fp16 tensor_tensor, 2x)
                nc.vector.tensor_tensor(bh[:], bh[:], tmp1[:], A.mult)
                base = T3_TILES.index(q) * 128
                nc.sync.dma_start(outh_d[base:base + 128, :], bh[:])

            for g in range(4):
                front(g)
            # interleaved emission, in-order queues in mind:
            #  DVE: chains first (unblock ACT squares ASAP), T3 tiles
            #       between chains, isles chase squares at the end
            #  ACT: window-sums for g0/g1 then 12 squares back-to-back
            chain(0, rem_engine="act")
            chain(1, rem_engine="act")
            sq0 = t1_sq(1, 0, 0)
            sq1 = t1_sq(1, 1, 1)
            chain(2, rem_engine="dve")
            sq2 = t1_sq(2, 0, 2)
            t3_tile(0, 0, 6)
            chain(3, rem_engine="dve")
            sq3 = t1_sq(2, 1, 3)
            sq4 = t1_sq(3, 0, 4)
            sq5 = t1_sq(3, 1, 5)
            t3_tile(0, 1, 7)
            t1_isle(1, 0, 0, sq0)
            t1_isle(1, 1, 1, sq1)
            t1_isle(2, 0, 2, sq2)
            t1_isle(2, 1, 3, sq3)
            t1_isle(3, 0, 4, sq4, split_store=True)
            t1_isle(3, 1, 5, sq5, split_store=True)

    nc.finalize()
    return nc


_CACHE: dict = {}


def _get_nc() -> bass.Bass:
    if "nc" not in _CACHE:
        _CACHE["nc"] = build_bass()
    return _CACHE["nc"]


def run(t, l, mask, trace: bool = False):
    t = np.ascontiguousarray(np.asarray(t, dtype=np.float32).reshape(B, 1))
    l = np.ascontiguousarray(np.asarray(l, dtype=np.float32).reshape(B, 1))
    mask = np.ascontiguousarray(np.asarray(mask, dtype=np.float32).reshape(B, L))
    p = np.arange(128, dtype=np.float32)[:, None]
    q = np.arange(NT, dtype=np.float32)[None, :]
    cbase = (q * 128 + p) * NPROBE
    idxh = np.tile(np.arange(HB, dtype=np.float16)[None, :], (128, 1))
    nc = _get_nc()
    in_maps = []
    for i in range(N_CORES):
        ts = t[i * ROWS:(i + 1) * ROWS].reshape(NT, 128).T
        ls = l[i * ROWS:(i + 1) * ROWS].reshape(NT, 128).T
        aux = np.ascontiguousarray(
            np.concatenate([ts, ls, cbase], axis=1), dtype=np.float32)
        in_maps.append({"mask": mask[i * ROWS:(i + 1) * ROWS], "aux": aux,
                        "idxh": idxh})
    res = run_bass_kernel_spmd(nc, in_maps, list(range(N_CORES)), trace=trace)
    out = np.empty((B, L), dtype=np.float32)
    for i in range(N_CORES):
        r0 = i * ROWS
        ob = np.asarray(res.results[i]["outb"])
        oh = np.asarray(res.results[i]["outh"])
        for ti, q_ in enumerate(T1_TILES):
            out[r0 + q_ * 128:r0 + (q_ + 1) * 128] = ob[ti * 128:(ti + 1) * 128]
        for ti, q_ in enumerate(T3_TILES):
            out[r0 + q_ * 128:r0 + (q_ + 1) * 128] = oh[ti * 128:(ti + 1) * 128]
    return out, res


def kernel(t, l, mask, length=None, **_unused) -> np.ndarray:
    out, _ = run(t, l, mask, trace=False)
    return out


# revision 3
# speedup vs baseline: 1.1417x; 1.1417x over previous
"""AttentionCrop Trainium2 kernel v3 (8 NeuronCores, data-parallel over batch).

Math (verified 0-wrong):  s = row-sum(mask), l_eff = max(l, s/2),
  a = t - l_eff, e = min(t + l_eff, s - 0.5); out[j] = (a <= j < e).

Measured-rate design (per [128, 4096] tile):
  T1 (tiles 0-5, u8 out):  ACT Square halves (2x 1.89us, fp16 idx input,
      per-partition bias 2048k - c) -> sq f32; DVE tensor_scalar
      is_le(sq, h|h|*1.0000003 + 0.2) -> u8 at 2x mode (2.26us).
  T3 (tiles 6-7, fp16 out): DVE per 2048-block: two fp16 4x compares
      (0.66us each) + one full-tile fp16 tensor_tensor mult at 2x (2.26us).
  Empty rows (a >= e): h < 0 so h|h| < 0 >= nothing passes; fp16 path
      compares fail likewise.
  s recovery: strided probes mask[:, 512k] k=2..7 -> c = ceil(s/512);
  512-wide SWDGE window at chunk c-1 -> exact remainder.  4 groups of
  2 tiles pipeline probes -> gather -> window-sum -> chain -> binarize.
  Window sums ride ACT Square accum_out (groups 0/1) or a DVE reduce
  (groups 2/3); chains are small [128,2] DVE ops.  Probes split across
  the sync and scalar HWDGE queues; stores ride sync.
  Measured: 56.8us (vs 75.1us v1 baseline), rel err 2.8e-4 (1 elem).
"""

import sys

import numpy as np

if "/opt/trn_rl_repo" not in sys.path:
    sys.path.insert(0, "/opt/trn_rl_repo")

import concourse.bacc as bacc
import concourse.bass as bass
import concourse.mybir as mybir
import concourse.tile as tile
from concourse.bass_utils import run_bass_kernel_spmd

N_CORES = 8
B, L = 8192, 4096
ROWS = B // N_CORES
NT = ROWS // 128
PROBE = 512
NPROBE = L // PROBE
KMIN = 2
NPR = NPROBE - KMIN
HB = 2048

F32 = mybir.dt.float32
F16 = mybir.dt.float16
U8 = mybir.dt.uint8
I32 = mybir.dt.int32
A = mybir.AluOpType
AF = mybir.ActivationFunctionType

# pairs pipeline; T3 tiles first (pure-DVE, no ACT dependency), then T1
PAIRS = ((6, 7), (0, 1), (2, 3), (4, 5))
T1_TILES = (0, 1, 2, 3, 4, 5)      # u8 out
T3_TILES = (6, 7)                  # fp16 out


def build_bass() -> bass.Bass:
    nc = bacc.Bacc()
    m_in = nc.declare_dram_parameter("mask", [ROWS, L], F32, isOutput=False)
    aux_in = nc.declare_dram_parameter("aux", [128, 3 * NT], F32, isOutput=False)
    idxh_in = nc.declare_dram_parameter("idxh", [128, HB], F16, isOutput=False)
    outb_d = nc.declare_dram_parameter("outb", [len(T1_TILES) * 128, L], U8,
                                       isOutput=True)
    outh_d = nc.declare_dram_parameter("outh", [len(T3_TILES) * 128, L], F16,
                                       isOutput=True)

    m_chunks = m_in.rearrange("r (k s) -> (r k) s", s=PROBE)
    m_probes = m_in.rearrange("(q p) (k s) -> p q k s", p=128, s=PROBE)

    with tile.TileContext(nc) as tc:
        with (
            tc.tile_pool(name="const", bufs=1) as cpool,
            tc.tile_pool(name="big", bufs=1) as bpool,
            tc.tile_pool(name="sq", bufs=6) as qpool,
            tc.tile_pool(name="small", bufs=1) as spool,
        ):
            aux = cpool.tile([128, 3 * NT], F32, tag="aux")
            t8 = aux[:, 0:NT]
            l8 = aux[:, NT:2 * NT]
            cb8 = aux[:, 2 * NT:3 * NT]

            idx_h = cpool.tile([128, HB], F16, tag="idxh")
            pr = cpool.tile([128, NT * NPR], F32, tag="pr")
            win = cpool.tile([128, NT * PROBE], F32, tag="win")
            junk = cpool.tile([128, PROBE], F32, tag="junk")
            tmp1 = cpool.tile([128, L], F16, tag="t3tmp", name="t3tmp")

            def small(tag, w=2, dt=F32):
                return spool.tile([128, w], dt, tag=tag, name=tag)

            prm = {g: {} for g in range(4)}

            # ---------------- emission ----------------
            # input DMAs on sync: probes pair-by-pair, aux + idx early
            for g, (q0, q1) in enumerate(PAIRS):
                eng = nc.sync if g < 2 else nc.scalar
                for q in (q0, q1):
                    eng.dma_start(
                        pr[:, q * NPR:(q + 1) * NPR],
                        m_probes[:, q, KMIN:NPROBE, 0],
                    )
                if g == 0:
                    nc.sync.dma_start(aux[:], aux_in[:, :])
                    nc.sync.dma_start(idx_h[:], idxh_in[:, :])

            def front(g):
                q0, q1 = PAIRS[g]
                cnt = small(f"cnt{g}")
                nc.vector.tensor_reduce(
                    cnt[:],
                    pr[:, q0 * NPR:(q1 + 1) * NPR].rearrange(
                        "p (q k) -> p q k", k=NPR),
                    axis=mybir.AxisListType.X, op=A.add,
                )
                wi = small(f"wi{g}", dt=I32)
                nc.vector.scalar_tensor_tensor(
                    wi[:], cnt[:], 1.0, cb8[:, q0:q1 + 1], A.add, A.add)
                for j, q in enumerate((q0, q1)):
                    nc.gpsimd.indirect_dma_start(
                        out=win[:, q * PROBE:(q + 1) * PROBE],
                        out_offset=None,
                        in_=m_chunks,
                        in_offset=bass.IndirectOffsetOnAxis(
                            ap=wi[:, j:j + 1], axis=0),
                    )
                prm[g]["cnt"] = cnt

            def chain(g, rem_engine="act"):
                q0, q1 = PAIRS[g]
                v = nc.vector
                rem = small(f"rem{g}")
                if rem_engine == "act":
                    for j, q in enumerate((q0, q1)):
                        nc.scalar.activation(
                            junk[:], win[:, q * PROBE:(q + 1) * PROBE],
                            AF.Square, accum_out=rem[:, j:j + 1])
                else:
                    v.tensor_reduce(
                        rem[:],
                        win[:, q0 * PROBE:(q1 + 1) * PROBE].rearrange(
                            "p (w s) -> p w s", s=PROBE),
                        axis=mybir.AxisListType.X, op=A.add,
                    )
                cnt = prm[g]["cnt"]
                sr = small(f"sr{g}")     # s - 512
                v.scalar_tensor_tensor(sr[:], cnt[:], float(PROBE), rem[:],
                                       A.mult, A.add)
                sh = small(f"sh{g}")     # s/2
                v.tensor_scalar(sh[:], sr[:], 0.5, float(PROBE) * 0.5,
                                A.mult, A.add)
                le = small(f"le{g}")
                v.tensor_tensor(le[:], sh[:], l8[:, q0:q1 + 1], A.max)
                a = small(f"a{g}")
                v.tensor_tensor(a[:], t8[:, q0:q1 + 1], le[:], A.subtract)
                e0 = small(f"e0{g}")
                v.tensor_tensor(e0[:], t8[:, q0:q1 + 1], le[:], A.add)
                sm = small(f"sm{g}")     # s - 0.5
                v.tensor_scalar(sm[:], sr[:], float(PROBE) - 0.5, None, A.add)
                e = small(f"e{g}")
                v.tensor_tensor(e[:], e0[:], sm[:], A.min)
                prm[g].update(a=a, e=e)
                if PAIRS[g][0] in T1_TILES:
                    # center form: negc_k = 2048k - c, hsm = h|h|*(1+3e-7)+0.2
                    cs = small(f"cs{g}")
                    v.tensor_tensor(cs[:], a[:], e[:], A.add)
                    negc = small(f"negc{g}")
                    v.tensor_scalar(negc[:], cs[:], -0.5, None, A.mult)
                    negc2 = small(f"negc2{g}")
                    v.tensor_scalar(negc2[:], negc[:], float(HB), None, A.add)
                    dd = small(f"dd{g}")
                    v.tensor_tensor(dd[:], e[:], a[:], A.subtract)
                    h = small(f"h{g}")
                    v.tensor_scalar(h[:], dd[:], 0.5, None, A.mult)
                    hneg = small(f"hneg{g}")
                    v.tensor_scalar(hneg[:], h[:], -1.0, None, A.mult)
                    habs = small(f"habs{g}")
                    v.tensor_tensor(habs[:], h[:], hneg[:], A.max)
                    hh = small(f"hh{g}")
                    v.tensor_tensor(hh[:], h[:], habs[:], A.mult)
                    hsm = small(f"hsm{g}")
                    v.tensor_scalar(hsm[:], hh[:], 1.0000003, 0.2,
                                    A.mult, A.add)
                    prm[g].update(negc=negc, negc2=negc2, hsm=hsm)

            def t1_sq(g, j, q):
                p = prm[g]
                sq = qpool.tile([128, L], F32, tag="sq", name=f"sq_{q}")
                nc.scalar.activation(sq[:, 0:HB], idx_h[:], AF.Square,
                                     bias=p["negc"][:, j:j + 1])
                nc.scalar.activation(sq[:, HB:L], idx_h[:], AF.Square,
                                     bias=p["negc2"][:, j:j + 1])
                return sq

            def t1_isle(g, j, q, sq, split_store=False):
                p = prm[g]
                bu = bpool.tile([128, L], U8, tag=f"b{q}", name=f"b_{q}")
                base = T1_TILES.index(q) * 128
                for k in range(2):
                    nc.vector.tensor_scalar(
                        bu[:, k * HB:(k + 1) * HB], sq[:, k * HB:(k + 1) * HB],
                        p["hsm"][:, j:j + 1], None, A.is_le)
                    if split_store:
                        nc.sync.dma_start(
                            outb_d[base:base + 128, k * HB:(k + 1) * HB],
                            bu[:, k * HB:(k + 1) * HB])
                if not split_store:
                    nc.sync.dma_start(outb_d[base:base + 128, :], bu[:])

            def t3_tile(g, j, q):
                p = prm[g]
                a = p["a"][:, j:j + 1]
                e = p["e"][:, j:j + 1]
                bh = bpool.tile([128, L], F16, tag=f"b{q}", name=f"b_{q}")
                for k in range(2):
                    # tmp1 block: j >= a ; bh block: j < e (both fp16 4x)
                    nc.vector.tensor_scalar(
                        tmp1[:, k * HB:(k + 1) * HB], idx_h[:], a,
                        -float(HB) * k, A.subtract, A.is_ge)
                    nc.vector.tensor_scalar(
                        bh[:, k * HB:(k + 1) * HB], idx_h[:], e,
                        -float(HB) * k, A.subtract, A.is_lt)
                # combine full tile (fp16 tensor_tensor, 2x)
                nc.vector.tensor_tensor(bh[:], bh[:], tmp1[:], A.mult)
                base = T3_TILES.index(q) * 128
                nc.sync.dma_start(outh_d[base:base + 128, :], bh[:])

            for g in range(4):
                front(g)
            # interleaved emission, in-order queues in mind:
            #  DVE: chains first (unblock ACT squares ASAP), T3 tiles
            #       between chains, isles chase squares at the end
            #  ACT: window-sums for g0/g1 then 12 squares back-to-back
            chain(0, rem_engine="act")
            chain(1, rem_engine="act")
            sq0 = t1_sq(1, 0, 0)
            sq1 = t1_sq(1, 1, 1)
            chain(2, rem_engine="dve")
            sq2 = t1_sq(2, 0, 2)
            t3_tile(0, 0, 6)
            chain(3, rem_engine="dve")
            sq3 = t1_sq(2, 1, 3)
            sq4 = t1_sq(3, 0, 4)
            sq5 = t1_sq(3, 1, 5)
            t3_tile(0, 1, 7)
            t1_isle(1, 0, 0, sq0)
            t1_isle(1, 1, 1, sq1)
            t1_isle(2, 0, 2, sq2)
            t1_isle(2, 1, 3, sq3)
            t1_isle(3, 0, 4, sq4, split_store=True)
            t1_isle(3, 1, 5, sq5, split_store=True)

    nc.finalize()
    return nc


_CACHE: dict = {}


def _get_nc() -> bass.Bass:
    if "nc" not in _CACHE:
        _CACHE["nc"] = build_bass()
    return _CACHE["nc"]


def run(t, l, mask, trace: bool = False):
    t = np.ascontiguousarray(np.asarray(t, dtype=np.float32).reshape(B, 1))
    l = np.ascontiguousarray(np.asarray(l, dtype=np.float32).reshape(B, 1))
    mask = np.ascontiguousarray(np.asarray(mask, dtype=np.float32).reshape(B, L))
    p = np.arange(128, dtype=np.float32)[:, None]
    q = np.arange(NT, dtype=np.float32)[None, :]
    cbase = (q * 128 + p) * NPROBE
    idxh = np.tile(np.arange(HB, dtype=np.float16)[None, :], (128, 1))
    nc = _get_nc()
    in_maps = []
    for i in range(N_CORES):
        ts = t[i * ROWS:(i + 1) * ROWS].reshape(NT, 128).T
        ls = l[i * ROWS:(i + 1) * ROWS].reshape(NT, 128).T
        aux = np.ascontiguousarray(
            np.concatenate([ts, ls, cbase], axis=1), dtype=np.float32)
        in_maps.append({"mask": mask[i * ROWS:(i + 1) * ROWS], "aux": aux,
                        "idxh": idxh})
    res = run_bass_kernel_spmd(nc, in_maps, list(range(N_CORES)), trace=trace)
    out = np.empty((B, L), dtype=np.float32)
    for i in range(N_CORES):
        r0 = i * ROWS
        ob = np.asarray(res.results[i]["outb"])
        oh = np.asarray(res.results[i]["outh"])
        for ti, q_ in enumerate(T1_TILES):
            out[r0 + q_ * 128:r0 + (q_ + 1) * 128] = ob[ti * 128:(ti + 1) * 128]
        for ti, q_ in enumerate(T3_TILES):
            out[r0 + q_ * 128:r0 + (q_ + 1) * 128] = oh[ti * 128:(ti + 1) * 128]
    return out, res


def kernel(t, l, mask, length=None, **_unused) -> np.ndarray:
    out, _ = run(t, l, mask, trace=False)
    return out
